# revision 1
# baseline (speedup 1.0000x reference)
"""EnhancedRGCN (3-layer GAT) Trainium2 kernel, 8-core SPMD.

Sharding: destination nodes across 8 cores. Host prep builds a static
padded-CSR (dst-degree-sorted windows of 128 nodes); gather indices into
the all-gathered node table and pad masks are uploaded once and reused
for all 3 layers. Per layer: PE node-side pipeline computes table rows
[h | a_s | a_d] = act(prev) @ Wbig, AllGather exchanges shards, the edge
phase gathers h|a_s per CSR slot column via indirect DMA and runs the
segment softmax + weighted aggregation with strided Vector/Scalar ops.
Softmax max-subtraction is skipped (shift invariance; bounded logits);
pad slots are masked to exp(-30) ~ 0.
"""

import sys

sys.path.insert(0, "/opt/trn_rl_repo")

import numpy as np

from concourse import bass, bacc, mybir, tile
from concourse.bass_utils import run_bass_kernel_spmd
from concourse.masks import make_identity

NC = 8
P = 128
F32 = mybir.dt.float32
ALU = mybir.AluOpType


def _host_prep(x, edge_index):
    N = x.shape[0]
    src = np.asarray(edge_index[0], dtype=np.int64)
    dst = np.asarray(edge_index[1], dtype=np.int64)

    npc = (N + NC - 1) // NC
    NW = (npc + P - 1) // P
    NP = NW * P
    TBL = NC * NP

    table_pos = np.empty(N, dtype=np.int64)
    perms = []
    for c in range(NC):
        lo, hi = c * npc, min((c + 1) * npc, N)
        n_loc = hi - lo
        deg = np.bincount(dst[(dst >= lo) & (dst < hi)] - lo, minlength=n_loc)
        order = np.argsort(-deg, kind="stable")
        perms.append(order + lo)
        table_pos[order + lo] = c * NP + np.arange(n_loc)

    cores = []
    for c in range(NC):
        lo, hi = c * npc, min((c + 1) * npc, N)
        n_loc = hi - lo
        emask = (dst >= lo) & (dst < hi)
        e_src, e_dst = src[emask], dst[emask] - lo
        rank_of_local = np.empty(n_loc, dtype=np.int64)
        rank_of_local[perms[c] - lo] = np.arange(n_loc)
        e_rank = rank_of_local[e_dst]
        deg_r = np.bincount(e_rank, minlength=NP)
        d_w = np.array([max(int(deg_r[w * P:(w + 1) * P].max()), 1)
                        for w in range(NW)])
        o = np.argsort(e_rank, kind="stable")
        e_rank_s, e_src_s = e_rank[o], e_src[o]
        slot = np.arange(len(e_rank_s)) - np.concatenate(
            [[0], np.cumsum(deg_r)])[e_rank_s]
        cores.append(dict(n_loc=n_loc, d_w=d_w, perm=perms[c],
                          e_rank=e_rank_s, e_src=e_src_s, slot=slot,
                          table_pos=table_pos))
    return cores, NW, NP, TBL, npc


def _build_program(NW, NP, TBL, d_w, S, Hs, slopes, scales):
    nc = bacc.Bacc("TRN2", target_bir_lowering=False, debug=False,
                   num_devices=NC)
    starts = np.concatenate([[0], np.cumsum(d_w)]).astype(int)

    x_sh = nc.dram_tensor("x_sh", [NP, 32], F32, kind="ExternalInput")
    idx_in = nc.dram_tensor("idx_in", [P, S], mybir.dt.int32, kind="ExternalInput")
    msk_in = nc.dram_tensor("msk_in", [P, S], F32, kind="ExternalInput")
    wb_in = nc.dram_tensor("wb_in", [32, 108], F32, kind="ExternalInput")
    bias_in = nc.dram_tensor("bias_in", [P, 96], F32, kind="ExternalInput")
    out_d = nc.dram_tensor("out_d", [NP, 32], F32, kind="ExternalOutput")

    tbl_sh = nc.dram_tensor("tbl_sh", [NP, 34], F32)
    tbl_full = nc.dram_tensor("tbl_full", [TBL, 34], F32, addr_space="Shared")

    with tile.TileContext(nc) as tc:
        with (
            tc.tile_pool(name="res", bufs=1) as res,
            tc.tile_pool(name="nodew", bufs=3) as nodew,
            tc.tile_pool(name="gat", bufs=3) as gat,
            tc.tile_pool(name="edgew", bufs=2) as edgew,
            tc.tile_pool(name="psum", bufs=2, space="PSUM") as psum,
            tc.tile_pool(name="psum2", bufs=2, space="PSUM") as psum2,
        ):
            ident = res.tile([P, P], F32)
            make_identity(nc, ident[:])
            idx_t = res.tile([P, S], mybir.dt.int32)
            nc.sync.dma_start(idx_t[:], idx_in[:])
            msk_t = res.tile([P, S], F32)
            nc.sync.dma_start(msk_t[:], msk_in[:])
            wb_t = res.tile([32, 108], F32)
            nc.sync.dma_start(wb_t[:], wb_in[:])
            bias_t = res.tile([P, 96], F32)
            nc.sync.dma_start(bias_t[:], bias_in[:])
            agg = res.tile([P, NW * 32], F32)
            a_d_res = res.tile([P, NW * 2], F32)

            for l in range(3):
                H = Hs[l]
                CH = 32 // H
                slope = float(slopes[l])
                # ---- node phase ----
                for w in range(NW):
                    xt = nodew.tile([P, 32], F32, tag="xt")
                    if l == 0:
                        nc.sync.dma_start(xt[:], x_sh[w * P:(w + 1) * P, :])
                    else:
                        nc.vector.tensor_tensor(
                            out=xt[:], in0=agg[:, w * 32:(w + 1) * 32],
                            in1=bias_t[:, (l - 1) * 32:l * 32], op=ALU.add)
                        if scales[l - 1] != 1.0:
                            nc.vector.tensor_scalar_mul(xt[:], xt[:],
                                                        float(scales[l - 1]))
                        tneg = nodew.tile([P, 32], F32, tag="tneg")
                        nc.vector.tensor_scalar_min(tneg[:], xt[:], 0.0)
                        nc.scalar.activation(tneg[:], tneg[:],
                                             mybir.ActivationFunctionType.Exp)
                        nc.vector.tensor_scalar_max(xt[:], xt[:], 0.0)
                        nc.vector.tensor_tensor(out=xt[:], in0=xt[:],
                                                in1=tneg[:], op=ALU.add)
                        nc.vector.tensor_scalar_add(xt[:], xt[:], -1.0)
                        nc.vector.tensor_scalar_min(xt[:], xt[:], 3.0)
                        nc.vector.tensor_scalar_max(xt[:], xt[:], -3.0)
                    pt = psum.tile([32, P], F32, tag="pt")
                    nc.tensor.transpose(out=pt[:], in_=xt[:], identity=ident[:])
                    xT = nodew.tile([32, P], F32, tag="xT")
                    nc.vector.tensor_copy(xT[:], pt[:])
                    pv = psum2.tile([P, 36], F32, tag="pv")
                    nc.tensor.matmul(pv[:], lhsT=xT[:],
                                     rhs=wb_t[:, l * 36:(l + 1) * 36],
                                     start=True, stop=True)
                    nv = nodew.tile([P, 36], F32, tag="nv")
                    nc.vector.tensor_copy(nv[:], pv[:])
                    nc.vector.tensor_copy(a_d_res[:, w * 2:w * 2 + H],
                                          nv[:, 32 + H:32 + 2 * H])
                    nc.sync.dma_start(tbl_sh[w * P:(w + 1) * P, :], nv[:, 0:34])
                # ---- exchange ----
                nc.gpsimd.collective_compute(
                    "AllGather", ALU.bypass,
                    replica_groups=[list(range(NC))],
                    ins=[tbl_sh.ap().opt()], outs=[tbl_full.ap().opt()],
                )
                # ---- edge phase ----
                for w in range(NW):
                    dw = int(d_w[w])
                    s0 = int(starts[w])
                    G = gat.tile([P, dw, 34], F32, tag="G")
                    for c in range(dw):
                        nc.gpsimd.indirect_dma_start(
                            out=G[:, c, :], out_offset=None, in_=tbl_full[:],
                            in_offset=bass.IndirectOffsetOnAxis(
                                ap=idx_t[:, s0 + c:s0 + c + 1], axis=0),
                        )
                    t = edgew.tile([P, 2, dw], F32, tag="t")
                    for h in range(H):
                        nc.vector.tensor_tensor(
                            out=t[:, h, :], in0=G[:, :, 32 + h],
                            in1=a_d_res[:, w * 2 + h:w * 2 + h + 1]
                                .to_broadcast([P, dw]),
                            op=ALU.add)
                    tv = t[:, 0:H, :]
                    u = edgew.tile([P, 2, dw], F32, tag="u")
                    nc.vector.tensor_scalar_mul(u[:, 0:H, :], tv, slope)
                    nc.vector.tensor_tensor(out=tv, in0=tv, in1=u[:, 0:H, :],
                                            op=ALU.max)
                    nc.vector.tensor_scalar_add(tv, tv, 30.0)
                    for h in range(H):
                        nc.vector.tensor_tensor(
                            out=t[:, h, :], in0=t[:, h, :],
                            in1=msk_t[:, s0:s0 + dw], op=ALU.mult)
                    nc.vector.tensor_scalar_add(tv, tv, -30.0)
                    nc.scalar.activation(tv, tv,
                                         mybir.ActivationFunctionType.Exp)
                    den = edgew.tile([P, 2], F32, tag="den")
                    nc.vector.tensor_reduce(den[:, 0:H], tv,
                                            mybir.AxisListType.X, ALU.add)
                    nc.vector.tensor_scalar_add(den[:, 0:H], den[:, 0:H], 1e-16)
                    rcp = edgew.tile([P, 2], F32, tag="rcp")
                    nc.vector.reciprocal(rcp[:, 0:H], den[:, 0:H])
                    nc.vector.tensor_tensor(
                        out=tv, in0=tv,
                        in1=rcp[:, 0:H].unsqueeze(2).to_broadcast([P, H, dw]),
                        op=ALU.mult)
                    tmp = edgew.tile([P, dw, 32], F32, tag="tmp")
                    for h in range(H):
                        nc.vector.tensor_tensor(
                            out=tmp[:, :, h * CH:(h + 1) * CH],
                            in0=G[:, :, h * CH:(h + 1) * CH],
                            in1=t[:, h, :].unsqueeze(2)
                                .to_broadcast([P, dw, CH]),
                            op=ALU.mult)
                    nc.vector.tensor_reduce(
                        agg[:, w * 32:(w + 1) * 32],
                        tmp[:].transpose([0, 2, 1]),
                        mybir.AxisListType.X, ALU.add)
            for w in range(NW):
                ot = nodew.tile([P, 32], F32, tag="ot")
                nc.vector.tensor_tensor(
                    out=ot[:], in0=agg[:, w * 32:(w + 1) * 32],
                    in1=bias_t[:, 64:96], op=ALU.add)
                nc.sync.dma_start(out_d[w * P:(w + 1) * P, :], ot[:])

    nc.compile()
    return nc


def kernel(x, edge_index, W1, att_s1, att_d1, b1, ea1,
           W2, att_s2, att_d2, b2, W3, att_s3, att_d3, b3):
    x = np.asarray(x, dtype=np.float32)
    Ws = [np.asarray(W1, np.float32), np.asarray(W2, np.float32),
          np.asarray(W3, np.float32)]
    att_ss = [np.asarray(att_s1, np.float32), np.asarray(att_s2, np.float32),
              np.asarray(att_s3, np.float32)]
    att_ds = [np.asarray(att_d1, np.float32), np.asarray(att_d2, np.float32),
              np.asarray(att_d3, np.float32)]
    bs = [np.asarray(b1, np.float32), np.asarray(b2, np.float32),
          np.asarray(b3, np.float32)]

    s = float(np.tanh(np.asarray(ea1, np.float32))[0])
    if s < 0.1:
        s = 1.0
    scales = [s * 1.05, 1.0, 1.0]
    Hs = [2, 2, 1]
    slopes = [0.01, 0.2, 0.2]

    N = x.shape[0]
    cores, NW, NP, TBL, npc = _host_prep(x, edge_index)

    d_w_u = np.max(np.stack([c["d_w"] for c in cores]), axis=0)
    S_u = int(d_w_u.sum())
    starts_u = np.concatenate([[0], np.cumsum(d_w_u)]).astype(int)

    # fused weight matrices [32, 36] each -> [32, 108]
    Wbigs = []
    for l in range(3):
        W, a_s, a_d = Ws[l], att_ss[l], att_ds[l]
        H = a_s.shape[0]
        CH = a_s.shape[1]
        M = np.zeros((32, 36), dtype=np.float32)
        M[:, :W.shape[0]] = W.T
        for h in range(H):
            M[:, 32 + h] = W.T[:, h * CH:(h + 1) * CH] @ a_s[h]
            M[:, 32 + H + h] = W.T[:, h * CH:(h + 1) * CH] @ a_d[h]
        Wbigs.append(M)
    wb_cat = np.concatenate(Wbigs, axis=1)
    bias_cat = np.tile(np.concatenate(bs)[None, :], (P, 1)).astype(np.float32)

    in_maps = []
    for c in range(NC):
        cc = cores[c]
        idx_u = np.zeros((P, S_u), dtype=np.int32)
        msk_u = np.zeros((P, S_u), dtype=np.float32)
        w_of = cc["e_rank"] // P
        col = starts_u[w_of] + cc["slot"]
        row = cc["e_rank"] % P
        idx_u[row, col] = cc["table_pos"][cc["e_src"]].astype(np.int32)
        msk_u[row, col] = 1.0
        x_pad = np.zeros((NP, 32), dtype=np.float32)
        x_pad[:cc["n_loc"]] = x[cc["perm"]]
        in_maps.append({"x_sh": x_pad, "idx_in": idx_u, "msk_in": msk_u,
                        "wb_in": wb_cat, "bias_in": bias_cat})

    nc = _build_program(NW, NP, TBL, d_w_u, S_u, Hs, slopes, scales)
    global LAST_EXEC_NS
    try:
        from concourse.timeline_sim import TimelineSim
        LAST_EXEC_NS = TimelineSim(nc, no_exec=True).simulate()
    except Exception:
        LAST_EXEC_NS = None
    res = run_bass_kernel_spmd(nc, in_maps, list(range(NC)))

    out = np.empty((N, 32), dtype=np.float32)
    for c in range(NC):
        cc = cores[c]
        out[cc["perm"]] = res.results[c]["out_d"][:cc["n_loc"]]
    return out



# revision 33
# speedup vs baseline: 2.3660x; 2.3660x over previous
"""EnhancedRGCN (3-layer GAT) Trainium2 kernel, 8-core SPMD.

Sharding: destination nodes across 8 cores. Host builds a static padded-CSR
(dst-degree-sorted windows of 128 nodes) whose columns are grouped into 4
src-core-pair blocks so the edge gather can use the hardware dma_gather
(int16 indices address a 2-shard 25088-row sub-table). Table rows live in a
Shared DRAM tensor, 64 floats per row (256B dma_gather granularity), in
[core][p][w] order. Per layer: node phase computes [h | a_s | a_d] with
block-diagonal PE matmuls, an AllGather exchanges the per-core tables,
then per window chunked dma_gathers (<=1024 indices each, the hardware
descriptor-ring limit) fetch the src rows; logits use a DVE leaky-relu and
Exp on the Scalar engine, with the weighted aggregation on Vector/GPSIMD.
Padding slots point at a pad table row (h=0, a_s=-3000): exp underflows
to 0 and h=0 kills the numerator, so no masking is needed.
"""

import os
import sys

sys.path.insert(0, "/opt/trn_rl_repo")

import numpy as np

from concourse import bass, bacc, mybir, tile
from concourse.bass_utils import run_bass_kernel_spmd
from concourse.masks import make_identity

NC = 8
P = 128
NG = 4              # src-core-pair groups
F32 = mybir.dt.float32
I16 = mybir.dt.int16
ALU = mybir.AluOpType
ACT = mybir.ActivationFunctionType

PAD_AS = -3000.0    # pad-row attention logit source value
POOL_MULT_FRAC = float(os.environ.get("POOL_MULT_FRAC", "0.0"))
NO_PRELU = os.environ.get("NO_PRELU", "1") == "1"
GCH = int(os.environ.get("GCH", "8"))


def _host_prep(x, edge_index):
    N = x.shape[0]
    src = np.asarray(edge_index[0], dtype=np.int64)
    dst = np.asarray(edge_index[1], dtype=np.int64)

    npc = (N + NC - 1) // NC
    NW = (npc + P - 1) // P
    NP = NW * P

    # rank of each node within its core (degree-sorted), and its table row
    # (within shard): row = (rank % P) * NW + rank // P
    rank_all = np.empty(N, dtype=np.int64)
    perms = []
    degs = []
    for c in range(NC):
        lo, hi = c * npc, min((c + 1) * npc, N)
        n_loc = hi - lo
        deg = np.bincount(dst[(dst >= lo) & (dst < hi)] - lo, minlength=n_loc)
        order = np.argsort(-deg, kind="stable")
        perms.append(order + lo)
        rank_of_local = np.empty(n_loc, dtype=np.int64)
        rank_of_local[order] = np.arange(n_loc)
        rank_all[lo:hi] = rank_of_local
        degs.append(deg)

    # per-core edge lists with (window, partition, pair-group, src)
    cores = []
    for c in range(NC):
        lo, hi = c * npc, min((c + 1) * npc, N)
        emask = (dst >= lo) & (dst < hi)
        e_src, e_dst = src[emask], dst[emask]
        e_rank = rank_all[e_dst]
        cores.append(dict(n_loc=hi - lo, perm=perms[c],
                          e_src=e_src, e_rank=e_rank))

    # unified per-(window, group) column widths across all cores
    Wg = np.zeros((NW, NG), dtype=np.int64)
    for c in range(NC):
        cc = cores[c]
        g = (cc["e_src"] // npc) // (NC // NG)
        w = cc["e_rank"] // P
        p = cc["e_rank"] % P
        cnt = np.zeros((NW, P, NG), dtype=np.int64)
        np.add.at(cnt, (w, p, g), 1)
        Wg = np.maximum(Wg, cnt.max(axis=1))
    Wg = np.maximum(Wg, 1)
    return cores, NW, NP, npc, rank_all, Wg


def _build_program(NW, NP, Wg, Hs, slopes, n_loc):
    nc = bacc.Bacc("TRN2", target_bir_lowering=False, debug=False,
                   num_devices=NC)
    TBL = NC * NP
    d_w2 = Wg.sum(axis=1).astype(int)          # window total columns
    starts2 = np.concatenate([[0], np.cumsum(d_w2)]).astype(int)
    S2 = int(d_w2.sum())
    dwmax = int(d_w2.max())

    assert 0 < NP - n_loc < P
    p_pad = P - (NP - n_loc)

    x_sh = nc.dram_tensor("x_sh", [P, NW, 32], F32, kind="ExternalInput")
    # wrapped int16 gather indices, one [128, 8*W] segment per (window, group)
    idx_in = nc.dram_tensor("idx_in", [P, 8 * S2], I16, kind="ExternalInput")
    wb_in = nc.dram_tensor("wb_in", [P, 324], F32, kind="ExternalInput")
    bias_in = nc.dram_tensor("bias_in", [P, 96], F32, kind="ExternalInput")
    out_d = nc.dram_tensor("out_d", [P, NW, 32], F32, kind="ExternalOutput")

    tbl_full = nc.dram_tensor("tbl_full", [TBL + P, 64], F32,
                              addr_space="Shared")
    barrier_in = nc.dram_tensor("barrier_in", [1, 8], F32)
    barrier_out = nc.dram_tensor("barrier_out", [NC, 8], F32,
                                 addr_space="Shared")
    barrier2_in = nc.dram_tensor("barrier2_in", [1, 8], F32)
    barrier2_out = nc.dram_tensor("barrier2_out", [NC, 8], F32,
                                  addr_space="Shared")

    frac = POOL_MULT_FRAC

    with tile.TileContext(nc) as tc:
        with (
            tc.tile_pool(name="res", bufs=1) as res,
            tc.tile_pool(name="xTp", bufs=2) as xTp,
            tc.tile_pool(name="ptp", bufs=2, space="PSUM") as ptp,
            tc.tile_pool(name="pvp", bufs=4, space="PSUM") as pvp,
            tc.tile_pool(name="gp", bufs=2) as gp,
            tc.tile_pool(name="ip", bufs=3) as ip,
            tc.tile_pool(name="tp", bufs=3) as tp,
            tc.tile_pool(name="ep", bufs=3) as ep,
            tc.tile_pool(name="tmpp", bufs=2) as tmpp,
        ):
            ident = res.tile([P, P], F32)
            make_identity(nc, ident[:])
            wb_t = res.tile([P, 324], F32)
            nc.sync.dma_start(wb_t[:], wb_in[:])
            bias_t = res.tile([P, 96], F32)
            nc.sync.dma_start(bias_t[:], bias_in[:])

            nv_all = res.tile([P, NW, 34], F32)
            ad_all = res.tile([P, NW, 2], F32)
            agg_all = res.tile([P, NW, 32], F32)
            xact_all = res.tile([P, NW, 32], F32)
            tneg = res.tile([P, NW, 32], F32)
            den_all = res.tile([P, NW, 2], F32)
            r_all = res.tile([P, NW, 2], F32)

            pid = nc.gpsimd.partition_id()
            shard = tbl_full[bass.ds(pid * NP, NP), :].rearrange(
                "(p w) f -> p w f", p=P)

            padc = res.tile([P - p_pad, 34], F32)
            nc.vector.memset(padc[:, 0:32], 0.0)
            nc.vector.memset(padc[:, 32:34], PAD_AS)
            nc.gpsimd.dma_start(shard[p_pad:P, NW - 1, 0:34], padc[:])

            nc.sync.dma_start(xact_all[:],
                              x_sh[:].rearrange("p w f -> p (w f)"))

            for l in range(3):
                H = Hs[l]
                CH = 32 // H
                slope = float(slopes[l])

                # ---- node phase ----
                if l > 0:
                    bslc = bias_t[:, (l - 1) * 32:l * 32]
                    nc.vector.tensor_tensor(
                        out=xact_all[:], in0=agg_all[:],
                        in1=bslc.unsqueeze(1).to_broadcast([P, NW, 32]),
                        op=ALU.add)
                    nc.vector.tensor_scalar_min(tneg[:], xact_all[:], 0.0)
                    nc.scalar.activation(tneg[:], tneg[:], ACT.Exp)
                    nc.vector.tensor_scalar_max(xact_all[:], xact_all[:], 0.0)
                    nc.vector.scalar_tensor_tensor(
                        out=xact_all[:], in0=tneg[:], scalar=-1.0,
                        in1=xact_all[:], op0=ALU.add, op1=ALU.add)
                    nc.vector.tensor_scalar(
                        out=xact_all[:], in0=xact_all[:],
                        scalar1=3.0, scalar2=-3.0,
                        op0=ALU.min, op1=ALU.max)

                for wb in range(0, NW, 3):
                    cc = min(3, NW - wb)
                    pt = ptp.tile([P, P], F32, tag="pt")
                    nc.tensor.transpose(out=pt[0:cc * 32, :],
                                        in_=xact_all[:, wb:wb + cc, :],
                                        identity=ident[:])
                    xT = xTp.tile([P, P], F32, tag="xT")
                    nc.vector.tensor_copy(xT[0:cc * 32, :], pt[0:cc * 32, :])
                    pv = pvp.tile([P, 108], F32, tag="pv")
                    nc.tensor.matmul(pv[:, 0:36 * cc],
                                     lhsT=xT[0:32 * cc, :],
                                     rhs=wb_t[0:32 * cc,
                                              108 * l:108 * l + 36 * cc],
                                     start=True, stop=True)
                    for wl in range(cc):
                        w = wb + wl
                        nc.vector.tensor_copy(nv_all[:, w, :],
                                              pv[:, 36 * wl:36 * wl + 34])
                        nc.scalar.copy(ad_all[:, w, 0:H],
                                       pv[:, 36 * wl + 32 + H:
                                          36 * wl + 32 + 2 * H])

                # ---- exchange: direct shard store + tiny barrier ----
                if l > 0:
                    # pre-store barrier: no core may overwrite its shard
                    # until every core finished reading the previous layer.
                    # Entry depends on this core's last edge-phase reduce
                    # (which transitively follows all its gathers).
                    nc.gpsimd.dma_start(barrier2_in[:],
                                        agg_all[0:1, NW - 1, 0:8])
                    nc.gpsimd.collective_compute(
                        "AllGather", ALU.bypass,
                        replica_groups=[list(range(NC))],
                        ins=[barrier2_in.ap().opt()],
                        outs=[barrier2_out.ap().opt()],
                    )
                    d2row = (p_pad + 2) * NW + NW - 1
                    nc.gpsimd.dma_start(
                        tbl_full[bass.ds(pid * NP + d2row, 1), 0:8],
                        barrier2_out[0:1, :])
                nc.gpsimd.dma_start(shard[:, 0:NW - 1, 0:34],
                                    nv_all[:, 0:NW - 1, :])
                nc.gpsimd.dma_start(shard[0:p_pad, NW - 1, 0:34],
                                    nv_all[0:p_pad, NW - 1, :])
                nc.gpsimd.dma_start(barrier_in[:],
                                    tbl_full[bass.ds(pid * NP, 1), 0:8])
                nc.gpsimd.collective_compute(
                    "AllGather", ALU.bypass,
                    replica_groups=[list(range(NC))],
                    ins=[barrier_in.ap().opt()],
                    outs=[barrier_out.ap().opt()],
                )
                # post-barrier token lands INSIDE each pair's gather slice
                # (an unused pad row) so every dma_gather depends on it
                dummy_row = (p_pad + 1) * NW + NW - 1
                for g in range(NG):
                    nc.gpsimd.dma_start(
                        tbl_full[g * 2 * NP + dummy_row:
                                 g * 2 * NP + dummy_row + 1, 0:8],
                        barrier_out[0:1, :])

                # ---- edge phase ----
                for w in range(NW):
                    dw = int(d_w2[w])
                    s0 = int(starts2[w])
                    G = gp.tile([P, dwmax, 64], F32, tag="G")
                    it = ip.tile([P, 8 * dwmax], I16, tag="it")
                    nc.sync.dma_start(it[:, 0:8 * dw],
                                      idx_in[:, 8 * s0:8 * (s0 + dw)])
                    co = 0
                    for g in range(NG):
                        Wc = int(Wg[w][g])
                        # chunk to <= GCH columns per gather (descriptor ring)
                        for o in range(0, Wc, GCH):
                            wc = min(GCH, Wc - o)
                            nc.gpsimd.dma_gather(
                                out_ap=G[:, co + o:co + o + wc, :],
                                in_ap=tbl_full[g * 2 * NP:
                                               (g + 1) * 2 * NP, :],
                                idxs_ap=it[:, 8 * (co + o):
                                           8 * (co + o + wc)],
                                num_idxs=128 * wc, num_idxs_reg=128 * wc,
                                elem_size=64)
                        co += Wc
                    gsl = G[:, 0:dw, :]
                    t = tp.tile([P, dwmax, 2], F32, tag="t")
                    e = ep.tile([P, dwmax, 2], F32, tag="e")
                    if NO_PRELU:
                        nc.vector.tensor_tensor(
                            out=t[:, 0:dw, 0:H], in0=gsl[:, :, 32:32 + H],
                            in1=ad_all[:, w, 0:H].unsqueeze(1)
                                .to_broadcast([P, dw, H]),
                            op=ALU.add)
                        nc.vector.scalar_tensor_tensor(
                            out=t[:, 0:dw, 0:H], in0=t[:, 0:dw, 0:H],
                            scalar=slope, in1=t[:, 0:dw, 0:H],
                            op0=ALU.mult, op1=ALU.max)
                    else:
                        for h in range(H):
                            nc.scalar.activation(
                                t[:, 0:dw, h], gsl[:, :, 32 + h], ACT.Prelu,
                                bias=ad_all[:, w, h:h + 1], alpha=slope)
                    nc.scalar.activation(e[:, 0:dw, 0:H], t[:, 0:dw, 0:H],
                                         ACT.Exp)
                    nc.vector.tensor_reduce(
                        den_all[:, w, 0:H],
                        e[:, 0:dw, 0:H].transpose([0, 2, 1]),
                        mybir.AxisListType.X, ALU.add)
                    tmp = tmpp.tile([P, dwmax, 32], F32, tag="tmp")
                    use_pool = (int((w + 1) * frac) - int(w * frac)) > 0
                    eng = nc.gpsimd if use_pool else nc.vector
                    for h in range(H):
                        eng.tensor_tensor(
                            out=tmp[:, 0:dw, h * CH:(h + 1) * CH],
                            in0=gsl[:, :, h * CH:(h + 1) * CH],
                            in1=e[:, 0:dw, h].unsqueeze(2)
                                .to_broadcast([P, dw, CH]),
                            op=ALU.mult)
                    nc.vector.tensor_reduce(
                        agg_all[:, w, :],
                        tmp[:, 0:dw, :].transpose([0, 2, 1]),
                        mybir.AxisListType.X, ALU.add)

                # ---- softmax normalization (batched) ----
                nc.vector.tensor_scalar_add(den_all[:, :, 0:H],
                                            den_all[:, :, 0:H], 1e-16)
                nc.vector.reciprocal(r_all[:, :, 0:H], den_all[:, :, 0:H])
                for h in range(H):
                    nc.vector.tensor_tensor(
                        out=agg_all[:, :, h * CH:(h + 1) * CH],
                        in0=agg_all[:, :, h * CH:(h + 1) * CH],
                        in1=r_all[:, :, h].unsqueeze(2)
                            .to_broadcast([P, NW, CH]),
                        op=ALU.mult)

            nc.vector.tensor_tensor(
                out=xact_all[:], in0=agg_all[:],
                in1=bias_t[:, 64:96].unsqueeze(1).to_broadcast([P, NW, 32]),
                op=ALU.add)
            nc.sync.dma_start(out_d[:].rearrange("p w f -> p (w f)"),
                              xact_all[:])

    nc.compile()
    return nc


def kernel(x, edge_index, W1, att_s1, att_d1, b1, ea1,
           W2, att_s2, att_d2, b2, W3, att_s3, att_d3, b3):
    x = np.asarray(x, dtype=np.float32)
    Ws = [np.asarray(W1, np.float32), np.asarray(W2, np.float32),
          np.asarray(W3, np.float32)]
    att_ss = [np.asarray(att_s1, np.float32), np.asarray(att_s2, np.float32),
              np.asarray(att_s3, np.float32)]
    att_ds = [np.asarray(att_d1, np.float32), np.asarray(att_d2, np.float32),
              np.asarray(att_d3, np.float32)]
    bs = [np.asarray(b1, np.float32), np.asarray(b2, np.float32),
          np.asarray(b3, np.float32)]

    s = float(np.tanh(np.asarray(ea1, np.float32))[0])
    if s < 0.1:
        s = 1.0
    c1 = s * 1.05
    Hs = [2, 2, 1]
    slopes = [0.01, 0.2, 0.2]

    N = x.shape[0]
    cores, NW, NP, npc, rank_all, Wg = _host_prep(x, edge_index)
    n_loc = cores[0]["n_loc"]
    assert all(c["n_loc"] == n_loc for c in cores)

    d_w2 = Wg.sum(axis=1).astype(int)
    starts2 = np.concatenate([[0], np.cumsum(d_w2)]).astype(int)
    S2 = int(d_w2.sum())
    goff = np.concatenate(
        [np.zeros((NW, 1), dtype=np.int64), np.cumsum(Wg, axis=1)], axis=1)

    # pad slots gather the even shard's first pad row (local row space)
    r_pad = n_loc
    pad_local = (r_pad % P) * NW + (r_pad // P)

    # block-diagonal fused weights [P, 324] (3 layers x 3-window blocks)
    wb_cat = np.zeros((P, 324), dtype=np.float32)
    for l in range(3):
        W, a_s, a_d = Ws[l], att_ss[l], att_ds[l]
        H = a_s.shape[0]
        CH = a_s.shape[1]
        M = np.zeros((32, 36), dtype=np.float32)
        M[:, :W.shape[0]] = W.T * (c1 if l == 0 else 1.0)
        for h in range(H):
            M[:, 32 + h] = W.T[:, h * CH:(h + 1) * CH] @ a_s[h]
            M[:, 32 + H + h] = W.T[:, h * CH:(h + 1) * CH] @ a_d[h]
        for i in range(3):
            wb_cat[32 * i:32 * (i + 1),
                   108 * l + 36 * i:108 * l + 36 * (i + 1)] = M
    bias_all = np.concatenate([bs[0] * c1, bs[1], bs[2]])
    bias_cat = np.tile(bias_all[None, :], (P, 1)).astype(np.float32)

    in_maps = []
    for c in range(NC):
        cc = cores[c]
        # linear slot index within each (w, g) block: i = col_local*128 + p
        e_src, e_rank = cc["e_src"], cc["e_rank"]
        w = e_rank // P
        p = e_rank % P
        src_core = e_src // npc
        g = src_core // (NC // NG)
        # slot counter within (w, p, g)
        order = np.lexsort((p, g, w))
        wo, po, go_ = w[order], p[order], g[order]
        key = (wo * P + po) * NG + go_
        # rank within same key (consecutive after sort)
        first = np.ones(len(key), dtype=bool)
        first[1:] = key[1:] != key[:-1]
        run_start = np.maximum.accumulate(np.where(first, np.arange(len(key)), 0))
        slot = np.arange(len(key)) - run_start
        # int16 value: (src_core % 2) * NP + table-row within shard
        srank = rank_all[e_src][order]
        val16 = ((src_core[order] % 2) * NP
                 + (srank % P) * NW + (srank // P)).astype(np.int16)
        # fill linear index lists per (w, g)
        lin = np.full((S2, P), pad_local, dtype=np.int16)  # [global col, p]
        gcol = starts2[wo] + goff[wo, go_] + slot
        lin[gcol, po] = val16
        # wrap: per (w, g) segment of n=128*W indices ordered i=(col*128+p):
        # wrapped[j%16 -> partition, j//16 -> free], replicated 8x
        idx16 = np.empty((P, 8 * S2), dtype=np.int16)
        for wdx in range(NW):
            for gg in range(NG):
                c0 = starts2[wdx] + goff[wdx, gg]
                Wc = int(Wg[wdx][gg])
                seg = lin[c0:c0 + Wc, :].reshape(-1)      # i = col*128+p
                wrapped = seg.reshape(-1, 16).T           # [16, n/16]
                idx16[:, 8 * c0:8 * (c0 + Wc)] = np.tile(wrapped, (8, 1))
        xp = x[cc["perm"]]
        xp = np.concatenate(
            [xp, np.zeros((NP - n_loc, 32), np.float32)], axis=0)
        x_pad = np.ascontiguousarray(
            xp.reshape(NW, P, 32).transpose(1, 0, 2))
        in_maps.append({"x_sh": x_pad, "idx_in": idx16,
                        "wb_in": wb_cat, "bias_in": bias_cat})

    nc = _build_program(NW, NP, Wg, Hs, slopes, n_loc)
    global LAST_EXEC_NS, LAST_NC
    LAST_NC = nc
    try:
        from concourse.timeline_sim import TimelineSim
        LAST_EXEC_NS = TimelineSim(nc, no_exec=True).simulate()
    except Exception:
        LAST_EXEC_NS = None
    if os.environ.get("BASS_BUILD_ONLY"):
        return None
    res = run_bass_kernel_spmd(nc, in_maps, list(range(NC)))

    out = np.empty((N, 32), dtype=np.float32)
    for c in range(NC):
        cc = cores[c]
        o = res.results[c]["out_d"]
        o = o.transpose(1, 0, 2).reshape(NP, 32)[:n_loc]
        out[cc["perm"]] = o
    return out
